# revision 25
# baseline (speedup 1.0000x reference)
"""Trainium2 Bass kernel for nn_Attention_86698209837214.

Multi-head attention: out = softmax(q k^T / 8) v @ W_out + b_out with
B=4, N=2048, DIM=1024, H=16, Dh=64, fp32.

Sharding: 8 cores = (batch b in 0..3) x (head-half hh in 0..1); each core
computes 8 heads of one batch. Host pre-transposes x[b] and slices weights;
host sums the per-core / per-head-pair partial outputs and adds b_out.

Device dataflow per core (all matmuls in float32r, full PE rate at N>=256):
  1. v = x @ Wv in natural [n, c] layout (lhsT = xT tiles).
  2. qT, kT = (x @ Wq/Wk)^T in [c, n] layout (lhsT = W tiles).
  3. Attention per head-pair hp (c-tile) and i-chunk (512 wide):
     dots^T [j, i] tiles per head via K=64 row-packed matmuls (2 j-tiles per
     PSUM tile so exp ops run at free dim 1024),
     exp on ScalarE (scale=1/8 folded; no max subtraction - logits ~N(0,1)),
     attn@v accumulated over j in PSUM with M=65 weights: v is augmented with
     a ones column, so row 64 of the accumulator is the softmax denominator.
     attn@v lags dots by one jt-pair so the PE never waits on the current exp.
     Normalize: denominator row -> partition 0 via DMA hop, fast reciprocal,
     gpsimd partition_broadcast, DVE multiply; the s=1 half reaches aT
     partitions 64:127 via an SBUF-to-SBUF DMA (engines cannot move data
     across partitions).
  4. Out-projection interleaved per (hp, ic); host sums per-pair partials.
     Measured on trn2: ~560-620 us per core span, rel err 3.2e-4 vs fp32.
"""

import sys

for _p in ("/opt/trn_rl_repo",):
    if _p not in sys.path:
        sys.path.append(_p)

from contextlib import ExitStack

import numpy as np

import concourse.bass as bass  # noqa: F401
import concourse.tile as tile
from concourse import bacc, mybir
from concourse.bass_utils import run_bass_kernel_spmd

F32 = mybir.dt.float32
F32R = mybir.dt.float32r
AF = mybir.ActivationFunctionType

P = 128
NSEQ = 2048  # sequence length per batch
D = 1024  # model dim
CH = 512  # per-core head-dim width (8 heads x 64)
DH = 64
NPAIR = 4  # head pairs per core (c-tiles of 128)
NDT = D // P  # 8 d-tiles
NNT = NSEQ // P  # 16 n-tiles
NNC = NSEQ // 512  # 4 n-chunks
SCALE = 0.125  # DIM_HEAD ** -0.5


def build_program():
    nc = bacc.Bacc("TRN2", target_bir_lowering=False, debug=False)

    xt = nc.dram_tensor("xt", [D, NSEQ], F32R, kind="ExternalInput")
    wqkv = nc.dram_tensor("wqkv", [D, 3 * CH], F32R, kind="ExternalInput")
    wout = nc.dram_tensor("wout", [CH, D], F32R, kind="ExternalInput")
    ones_in = nc.dram_tensor("ones", [P, 1], F32R, kind="ExternalInput")
    out = nc.dram_tensor("out", [NPAIR, NSEQ, D], F32, kind="ExternalOutput")

    xt_t = xt.ap().rearrange("(dt p) n -> dt p n", p=P)  # [8, 128, 2048]
    wqkv_t = wqkv.ap().rearrange("(dt p) c -> dt p c", p=P)  # [8, 128, 1536]
    wout_t = wout.ap().rearrange("(ct p) e -> ct p e", p=P)  # [4, 128, 1024]
    out_t = out.ap().rearrange("hp (nt p) e -> hp nt p e", p=P)  # [4, 16, 128, 1024]

    with tile.TileContext(nc) as tc, ExitStack() as ctx:
        # ---- persistent pools (whole kernel) ----
        p_qk = ctx.enter_context(tc.tile_pool(name="p_qk", bufs=1))  # 64 KB/p
        p_v = ctx.enter_context(tc.tile_pool(name="p_v", bufs=1))  # 32 KB/p
        p_small = ctx.enter_context(tc.tile_pool(name="p_small", bufs=1))
        # PSUM: mm 2x[128,1024] (4 banks) + av 3x[65,512] (3) + out (1) = 8 banks
        ps_mm = ctx.enter_context(tc.tile_pool(name="ps_mm", bufs=2, space="PSUM"))
        ps_av = ctx.enter_context(tc.tile_pool(name="ps_av", bufs=3, space="PSUM"))
        ps_out = ctx.enter_context(tc.tile_pool(name="ps_out", bufs=1, space="PSUM"))

        ones = p_small.tile([P, 1], F32R, tag="ones")
        nc.sync.dma_start(out=ones, in_=ones_in.ap())
        # dummy exp: pulls the ~2.7us ACT_TABLE_LOAD for the Exp set into the
        # initial DMA wait instead of the first real softmax tile
        warm = p_small.tile([P, 1], F32, tag="warm")
        nc.scalar.activation(out=warm, in_=ones.bitcast(F32), func=AF.Exp, scale=1.0)

        # ---- phase A: load xt, wv, wk; compute v_aug ----
        st_xt = ExitStack()
        p_xt = st_xt.enter_context(tc.tile_pool(name="p_xt", bufs=1))  # 64 KB/p
        st_wk = ExitStack()
        p_wk = st_wk.enter_context(tc.tile_pool(name="p_wk", bufs=1))  # 16 KB/p
        st_wv = ExitStack()
        p_wv = st_wv.enter_context(tc.tile_pool(name="p_wv", bufs=1))  # 16 KB/p

        xt_tiles = []
        wv_tiles = []
        wk_tiles = []
        for dt_i in range(NDT):
            t = p_xt.tile([P, NSEQ], F32R, tag=f"xt{dt_i}")
            nc.sync.dma_start(out=t, in_=xt_t[dt_i])
            xt_tiles.append(t)
            t = p_wv.tile([P, CH], F32R, tag=f"wv{dt_i}")
            nc.sync.dma_start(out=t, in_=wqkv_t[dt_i][:, 2 * CH : 3 * CH])
            wv_tiles.append(t)
            t = p_wk.tile([P, CH], F32R, tag=f"wk{dt_i}")
            nc.sync.dma_start(out=t, in_=wqkv_t[dt_i][:, CH : 2 * CH])
            wk_tiles.append(t)

        # v_aug: per head-slot sg, 65 cols = [v_sg (64) | ones (1)]; the ones
        # column makes the attn@v matmul also produce the softmax denominator.
        # Allocate all v tiles upfront and write the ones columns first so the
        # per-tile critical path is just matmuls + copies.
        v_tiles = []
        for nt in range(NNT):
            dst = p_v.tile([P, 8 * 65], F32R, tag=f"v{nt}")
            ones_dst = dst.rearrange("p (h c) -> p h c", c=65)[:, :, 64:65]
            nc.gpsimd.dma_start(out=ones_dst, in_=ones_in.ap().to_broadcast([P, 8, 1]))
            v_tiles.append(dst)
        for nt in range(NNT):
            dst = v_tiles[nt]
            acc = ps_mm.tile([P, 512], F32, tag="mm")
            for dt_i in range(NDT):
                nc.tensor.matmul(
                    acc,
                    xt_tiles[dt_i][:, nt * P : (nt + 1) * P],
                    wv_tiles[dt_i],
                    start=(dt_i == 0),
                    stop=(dt_i == NDT - 1),
                )
            v_dst = dst.rearrange("p (h c) -> p h c", c=65)[:, :, 0:DH]
            nc.vector.tensor_copy(v_dst, acc.rearrange("p (h c) -> p h c", c=DH))
        st_wv.close()

        # ---- phase B: kT c-tiles (wk), then qT c-tiles (wq prefetched) ----
        st_wq = ExitStack()
        p_wq = st_wq.enter_context(tc.tile_pool(name="p_wq", bufs=1))  # 16 KB/p
        wq_tiles = []
        for dt_i in range(NDT):
            t = p_wq.tile([P, CH], F32R, tag=f"wq{dt_i}")
            nc.sync.dma_start(out=t, in_=wqkv_t[dt_i][:, 0:CH])
            wq_tiles.append(t)

        kT_tiles = []
        qT_tiles = []
        for which, w_tiles in (("k", wk_tiles), ("q", wq_tiles)):
            for ct in range(NPAIR):
                dst = p_qk.tile([P, NSEQ], F32R, tag=f"{which}T{ct}")
                woff = ct * P
                for nch in range(NNC):
                    acc = ps_mm.tile([P, 512], F32, tag="mm")
                    for dt_i in range(NDT):
                        nc.tensor.matmul(
                            acc,
                            w_tiles[dt_i][:, woff : woff + P],
                            xt_tiles[dt_i][:, nch * 512 : (nch + 1) * 512],
                            start=(dt_i == 0),
                            stop=(dt_i == NDT - 1),
                        )
                    nc.vector.tensor_copy(dst[:, nch * 512 : (nch + 1) * 512], acc)
                (kT_tiles if which == "k" else qT_tiles).append(dst)
        st_wq.close()
        st_wk.close()
        st_xt.close()

        # ---- attention-phase pools (reuse xt/w space) ----
        p_exp = ctx.enter_context(tc.tile_pool(name="p_exp", bufs=10))  # 40 KB/p
        p_aT = ctx.enter_context(tc.tile_pool(name="p_aT", bufs=2))  # 16 KB/p
        p_wout = ctx.enter_context(tc.tile_pool(name="p_wout", bufs=1))  # 16 KB/p
        p_den = ctx.enter_context(tc.tile_pool(name="p_den", bufs=1))
        p_recip = ctx.enter_context(tc.tile_pool(name="p_recip", bufs=1))
        p_bcast = ctx.enter_context(tc.tile_pool(name="p_bcast", bufs=2))
        p_ostage = ctx.enter_context(tc.tile_pool(name="p_ostage", bufs=3))

        wout_tiles = []
        for ct in range(NPAIR):
            t = p_wout.tile([P, D], F32R, tag=f"wout{ct}")
            nc.gpsimd.dma_start(out=t, in_=wout_t[ct])
            wout_tiles.append(t)

        # ---- phase C: attention; out-projection interleaved per (hp, ic) ----
        NJP = NNT // 2
        for hp in range(NPAIR):
            aT = p_aT.tile([P, NSEQ], F32R, tag="aT")
            for ic in range(NNC):
                i0 = ic * 512
                av_ps = []
                for s in range(2):
                    av_s = ps_av.tile([65, 512], F32, tag="av", name=f"av{s}")
                    av_ps.append(av_s)

                def emit_av(jp, exp_pair):
                    for s in range(2):
                        sg = hp * 2 + s
                        for half in range(2):
                            jtx = 2 * jp + half
                            nc.tensor.matmul(
                                av_ps[s],
                                v_tiles[jtx][:, sg * 65 : sg * 65 + 65],
                                exp_pair[s][:, half * 512 : (half + 1) * 512],
                                start=(jp == 0 and half == 0),
                                stop=(jp == NJP - 1 and half == 1),
                            )

                prev_exp = None
                for jp in range(NJP):
                    exp_tiles = []
                    for s in range(2):
                        r0 = s * DH
                        dots = ps_mm.tile([P, 1024], F32, tag="mm")
                        for half in range(2):
                            jtx = 2 * jp + half
                            nc.tensor.matmul(
                                dots[:, half * 512 : (half + 1) * 512],
                                kT_tiles[hp][r0 : r0 + DH, jtx * P : (jtx + 1) * P],
                                qT_tiles[hp][r0 : r0 + DH, i0 : i0 + 512],
                                start=True,
                                stop=True,
                                tile_position=(r0, 0),
                            )
                        e = p_exp.tile([P, 1024], F32R, tag="exp")
                        nc.scalar.activation(out=e, in_=dots, func=AF.Exp, scale=SCALE)
                        exp_tiles.append(e)
                    # lag attn@v one jp behind dots: PE never waits on this
                    # iteration's exp
                    if prev_exp is not None:
                        emit_av(jp - 1, prev_exp)
                    prev_exp = exp_tiles
                emit_av(NJP - 1, prev_exp)

                # epilogue: rows 0:64 = unnormalized attn-out, row 64 = denom.
                # Cross-partition moves go through DMA; custom-DVE/gpsimd ops
                # only operate at partition base 0 (HW bug at nonzero bases).
                den_hi = p_den.tile([65, 1024], F32, tag="den_hi")
                for s in range(2):
                    nc.vector.tensor_copy(
                        den_hi[64:65, s * 512 : (s + 1) * 512], av_ps[s][64:65, :]
                    )
                den_sb = p_den.tile([1, 1024], F32, tag="den_sb")
                nc.gpsimd.dma_start(out=den_sb, in_=den_hi[64:65, :])
                recip = p_recip.tile([1, 1024], F32, tag="recip")
                nc.vector.reciprocal_approx_fast(out=recip, in_=den_sb)
                bcast = []
                for s in range(2):
                    bc = p_bcast.tile([DH, 512], F32, tag="bcast", name=f"bc{s}")
                    nc.gpsimd.partition_broadcast(
                        out_ap=bc, in_ap=recip[:, s * 512 : (s + 1) * 512]
                    )
                    bcast.append(bc)
                nc.vector.tensor_mul(
                    aT[0:DH, i0 : i0 + 512], av_ps[0][0:DH, :], bcast[0]
                )
                tmp = p_bcast.tile([DH, 512], F32R, tag="tmp")
                nc.vector.tensor_mul(tmp, av_ps[1][0:DH, :], bcast[1])
                nc.gpsimd.dma_start(out=aT[DH:P, i0 : i0 + 512], in_=tmp)

                # out-projection for this chunk's n-tiles
                for nt in range(4 * ic, 4 * ic + 4):
                    for ec in range(2):
                        o_ps = ps_out.tile([P, 512], F32, tag="o")
                        nc.tensor.matmul(
                            o_ps,
                            aT[:, nt * P : (nt + 1) * P],
                            wout_tiles[hp][:, ec * 512 : (ec + 1) * 512],
                            start=True,
                            stop=True,
                        )
                        o_sb = p_ostage.tile([P, 512], F32, tag="o_sb")
                        nc.vector.tensor_copy(o_sb, o_ps)
                        nc.sync.dma_start(
                            out=out_t[hp][nt][:, ec * 512 : (ec + 1) * 512], in_=o_sb
                        )

    nc.compile()
    return nc


_NC = None


def _get_program():
    global _NC
    if _NC is None:
        _NC = build_program()
    return _NC


INNER = 1024


def kernel(x, W_qkv, W_out, b_out):
    x = np.asarray(x, dtype=np.float32)
    W_qkv = np.asarray(W_qkv, dtype=np.float32)
    W_out = np.asarray(W_out, dtype=np.float32)
    b_out = np.asarray(b_out, dtype=np.float32)
    B = x.shape[0]

    nc = _get_program()
    in_maps = []
    for b in range(B):
        for hh in range(2):
            cs = hh * CH
            wq = W_qkv[:, cs : cs + CH]
            wk = W_qkv[:, INNER + cs : INNER + cs + CH]
            wv = W_qkv[:, 2 * INNER + cs : 2 * INNER + cs + CH]
            in_maps.append(
                {
                    "xt": np.ascontiguousarray(x[b].T),
                    "wqkv": np.ascontiguousarray(np.concatenate([wq, wk, wv], axis=1)),
                    "wout": np.ascontiguousarray(W_out[cs : cs + CH, :]),
                    "ones": np.ones((P, 1), dtype=np.float32),
                }
            )
    res = run_bass_kernel_spmd(nc, in_maps, core_ids=list(range(8)))
    out = np.empty((B, NSEQ, D), dtype=np.float32)
    for b in range(B):
        out[b] = (
            res.results[2 * b]["out"].sum(axis=0)
            + res.results[2 * b + 1]["out"].sum(axis=0)
            + b_out
        )
    return out
